# revision 6
# baseline (speedup 1.0000x reference)
"""Trainium2 Bass kernel for nn_End2End_10316511445013 (embedding_lookup).

Math being implemented (see the reference nn.Module):
  1. x = logits + g,  g = -ln(-ln(u))          [B,L,V]
  2. In fp32 the straight-through one-hot  y = y_hard + y_soft - y_soft  is
     *exactly* alpha * one_hot(argmax(x)) with alpha = fl(fl(1+s)-s) = 1 +/- 2^-23,
     so the einsum with the embedding table is exactly an embedding row gather
     scaled by alpha (~1, error < 1.2e-7 relative -> we use 1).
  3. inputs_embeds[b,l] = att[b,l] * (idx < AV) * W[idx],  idx = argmax_v x[b,l,:]
  4. psg path: trunc_ids / flag index logic on [B,L] int tensors, then a second
     row gather of W, all computed on-device with small DVE ops + indirect DMA.

Distribution: data-parallel over the B*L = 2048 rows; 256 rows per core; the
94MB embedding table is replicated to every core.  Per core each 128-row
group streams gumbel in chunks (4016 wide, with a narrow 2008/1004/1004
tail to shrink the drain); ACT computes b = Ln(Ln(u)*-1) in place, gpsimd
negates it (1-input op, ~line rate), and the logits chunk is then DMAd
*onto* the tile with accum_op=add (SDMA CCE inline add), yielding
x = logits - b with no DVE pass.  DVE only does the per-chunk row max
(tensor_reduce) and within-chunk argmax (max_index) -- 2 passes, under the
DMA roofline.  Group endgame picks the winning chunk with a [P,NCH]
max_index and masked-sum gathers of the stored per-chunk indices/bases.
"""

import os
import sys
import tempfile

import numpy as np

sys.path.insert(0, "/opt/trn_rl_repo")

B, L, V, AV, D = 4, 512, 32128, 32000, 768
R = B * L            # 2048 tokens total
NCORES = 8
RC = R // NCORES     # 256 tokens per core
P = 128              # partitions
GROUPS = RC // P     # 2 groups of 128 tokens
CW = 4016            # main chunk width
# 7 x 4016 + 2008 + 1004 + 1004 = 32128; narrow tail chunks shrink the drain
CHUNKS = [(k * CW, CW) for k in range(7)] + [
    (7 * CW, 2008),
    (7 * CW + 2008, 1004),
    (7 * CW + 3012, 1004),
]
NCH = len(CHUNKS)    # 10
NEG_BIG = -3.0e38

_CACHE = {}
LAST = {}            # exec_time_ns etc. for test harness introspection


def _build_program():
    from contextlib import ExitStack

    import concourse.bass as bass
    import concourse.tile as tile
    from concourse import bacc, mybir

    f32 = mybir.dt.float32
    i32 = mybir.dt.int32
    u32 = mybir.dt.uint32
    Alu = mybir.AluOpType
    Act = mybir.ActivationFunctionType

    nc = bacc.Bacc(
        "TRN2",
        target_bir_lowering=False,
        debug=False,
        enable_asserts=True,
        num_devices=NCORES,
    )

    lg_d = nc.dram_tensor("logits", [RC, V], f32, kind="ExternalInput")
    gu_d = nc.dram_tensor("gumbel", [RC, V], f32, kind="ExternalInput")
    w_d = nc.dram_tensor("wemb", [AV, D], f32, kind="ExternalInput")
    att_d = nc.dram_tensor("att", [B, L], i32, kind="ExternalInput")
    psg_d = nc.dram_tensor("psg", [B, L], i32, kind="ExternalInput")
    li_d = nc.dram_tensor("liota", [B, L], i32, kind="ExternalInput")
    bc_d = nc.dram_tensor("bcol", [RC, 1], i32, kind="ExternalInput")
    lc_d = nc.dram_tensor("lcol", [RC, 1], i32, kind="ExternalInput")
    am_d = nc.dram_tensor("attmy", [RC, 1], i32, kind="ExternalInput")
    out_d = nc.dram_tensor("out", [RC, D], f32, kind="ExternalOutput")
    sc2_d = nc.dram_tensor("scratch2", [B, 2], i32, kind="Internal")

    att_flat = att_d.ap().rearrange("b (l o) -> (b l) o", o=1)
    psg_flat = psg_d.ap().rearrange("b (l o) -> (b l) o", o=1)

    with tile.TileContext(nc) as tc, ExitStack() as ctx:
        sm = ctx.enter_context(tc.tile_pool(name="small", bufs=1))
        up = ctx.enter_context(tc.tile_pool(name="gu", bufs=3))
        lp = ctx.enter_context(tc.tile_pool(name="lg", bufs=4))
        ep = ctx.enter_context(tc.tile_pool(name="emb", bufs=1))
        tp = ctx.enter_context(tc.tile_pool(name="tok", bufs=2))
        mp = ctx.enter_context(tc.tile_pool(name="mi", bufs=2))

        # ---------------- psg index stage on [B, 512] ----------------
        A_t = sm.tile([B, L], i32, tag="psgA")
        nc.sync.dma_start(A_t[:], att_d.ap())
        P_t = sm.tile([B, L], i32, tag="psgP")
        nc.sync.dma_start(P_t[:], psg_d.ap())
        LI_t = sm.tile([B, L], i32, tag="psgLI")
        nc.sync.dma_start(LI_t[:], li_d.ap())

        shift = sm.tile([B, 1], i32, tag="shift")
        with nc.allow_low_precision(reason="exact int32 sum of 0/1 mask"):
            nc.vector.tensor_reduce(shift[:], A_t[:], mybir.AxisListType.X, Alu.add)

        FA = sm.tile([B, L], i32, tag="FA")  # FA[j] = att[511-j]
        nc.vector.tensor_copy(FA[:], A_t[:, ::-1])
        PR = sm.tile([B, L], i32, tag="PR")  # roll(psg,1) with [:,0]=1
        nc.vector.memset(PR[:, 0:1], 1)
        nc.vector.tensor_copy(PR[:, 1:L], P_t[:, 0 : L - 1])

        t1 = sm.tile([B, L], i32, tag="t1")
        nc.vector.tensor_scalar(t1[:], FA[:], 0, None, Alu.is_equal)
        t2 = sm.tile([B, L], i32, tag="t2")
        nc.vector.tensor_scalar(t2[:], PR[:], 0, None, Alu.not_equal)
        nzm = sm.tile([B, L], i32, tag="nzm")
        nc.vector.tensor_tensor(nzm[:], t1[:], t2[:], Alu.mult)

        # v(j) = (j + shift) & 511 : position in trunc space
        c511b = sm.tile([B, 1], i32, tag="c511b")
        nc.vector.memset(c511b[:], 511)
        v_t = sm.tile([B, L], i32, tag="v")
        nc.vector.tensor_tensor(
            v_t[:], LI_t[:], shift[:, 0:1].to_broadcast([B, L]), Alu.add
        )
        nc.vector.tensor_tensor(
            v_t[:], v_t[:], c511b[:, 0:1].to_broadcast([B, L]), Alu.bitwise_and
        )
        # cand = nz ? v : 9999  ==  (v - 9999)*nz + 9999
        c1 = sm.tile([B, L], i32, tag="c1")
        nc.vector.scalar_tensor_tensor(c1[:], v_t[:], 9999, nzm[:], Alu.subtract, Alu.mult)
        cand = sm.tile([B, L], i32, tag="cand")
        nc.vector.tensor_scalar(cand[:], c1[:], 9999, None, Alu.add)
        nzpos = sm.tile([B, 1], i32, tag="nzpos")
        nc.vector.tensor_reduce(nzpos[:], cand[:], mybir.AxisListType.X, Alu.min)

        s2t = sm.tile([B, 2], i32, tag="s2t")
        nc.vector.tensor_copy(s2t[:, 0:1], shift[:])
        nc.vector.tensor_copy(s2t[:, 1:2], nzpos[:])
        nc.sync.dma_start(sc2_d.ap(), s2t[:])

        ones_i = sm.tile([P, 1], i32, tag="ones")
        nc.vector.memset(ones_i[:], 1)
        c511p = sm.tile([P, 1], i32, tag="c511p")
        nc.vector.memset(c511p[:], 511)
        iotaN = sm.tile([P, NCH], i32, tag="iotaN")
        basesN = sm.tile([P, NCH], i32, tag="basesN")
        for j, (off, _w) in enumerate(CHUNKS):
            nc.vector.memset(iotaN[:, j : j + 1], j)
            nc.vector.memset(basesN[:, j : j + 1], off)

        # ---------------- early psg token-side gathers (independent of phase A) --
        e2s, s2fs, s1parts = [], [], []
        for g in range(GROUPS):
            rows = slice(g * P, (g + 1) * P)
            bvec = tp.tile([P, 1], i32, tag="bvec")
            nc.sync.dma_start(bvec[:], bc_d.ap()[rows, :])
            lvec = tp.tile([P, 1], i32, tag="lvec")
            nc.sync.dma_start(lvec[:], lc_d.ap()[rows, :])
            sn = tp.tile([P, 2], i32, tag="sn")
            nc.gpsimd.indirect_dma_start(
                out=sn[:],
                out_offset=None,
                in_=sc2_d.ap(),
                in_offset=bass.IndirectOffsetOnAxis(ap=bvec[:, 0:1], axis=0),
            )
            # p = (l - shift + 512) & 511
            pv = tp.tile([P, 1], i32, tag="pv")
            nc.vector.tensor_tensor(pv[:], lvec[:], sn[:, 0:1], Alu.subtract)
            nc.vector.tensor_scalar(pv[:], pv[:], 512, None, Alu.add)
            nc.vector.tensor_tensor(pv[:], pv[:], c511p[:], Alu.bitwise_and)
            bsh = tp.tile([P, 1], i32, tag="bsh")
            nc.vector.tensor_scalar(bsh[:], bvec[:], 512, None, Alu.mult)
            # gather att[b, 511-p] : off = b*512 + 511 - p
            offa2 = tp.tile([P, 1], i32, tag="offa2")
            nc.vector.tensor_scalar(offa2[:], pv[:], -1, 511, Alu.mult, Alu.add)
            nc.vector.tensor_tensor(offa2[:], offa2[:], bsh[:], Alu.add)
            gA = tp.tile([P, 1], i32, tag="gA")
            nc.gpsimd.indirect_dma_start(
                out=gA[:],
                out_offset=None,
                in_=att_flat,
                in_offset=bass.IndirectOffsetOnAxis(ap=offa2[:, 0:1], axis=0),
            )
            # gather psg_input[b, p-1] (clamped; p==0 handled by select)
            offp = tp.tile([P, 1], i32, tag="offp")
            nc.vector.tensor_tensor(offp[:], bsh[:], pv[:], Alu.add)
            nc.vector.tensor_scalar(offp[:], offp[:], -1, 0, Alu.add, Alu.max)
            gP = tp.tile([P, 1], i32, tag="gP")
            nc.gpsimd.indirect_dma_start(
                out=gP[:],
                out_offset=None,
                in_=psg_flat,
                in_offset=bass.IndirectOffsetOnAxis(ap=offp[:, 0:1], axis=0),
            )
            eq0 = tp.tile([P, 1], i32, tag="eq0")
            nc.vector.tensor_scalar(eq0[:], pv[:], 0, None, Alu.is_equal)
            gPe = tp.tile([P, 1], i32, tag="gPe")
            nc.vector.select(gPe[:], eq0[:], ones_i[:], gP[:])
            tA = tp.tile([P, 1], i32, tag="tA")
            nc.vector.tensor_scalar(tA[:], gA[:], -1, 1, Alu.mult, Alu.add)
            id2 = tp.tile([P, 1], i32, tag="id2")
            nc.vector.tensor_tensor(id2[:], tA[:], gPe[:], Alu.mult)
            s2f = sm.tile([P, 1], f32, tag=f"s2f{g}")
            nc.vector.tensor_tensor(s2f[:], lvec[:], sn[:, 1:2], Alu.is_ge)
            e2 = sm.tile([P, D], f32, tag=f"e2_{g}")
            nc.gpsimd.indirect_dma_start(
                out=e2[:],
                out_offset=None,
                in_=w_d.ap(),
                in_offset=bass.IndirectOffsetOnAxis(ap=id2[:, 0:1], axis=0),
            )
            am_t = tp.tile([P, 1], i32, tag="am")
            nc.sync.dma_start(am_t[:], am_d.ap()[rows, :])
            attf = sm.tile([P, 1], f32, tag=f"attf{g}")
            nc.vector.tensor_copy(attf[:], am_t[:])
            e2s.append(e2)
            s2fs.append(s2f)
            s1parts.append(attf)

        # ---------------- phase A: stream chunks group-sequentially ----------------
        for g in range(GROUPS):
            rows = slice(g * P, (g + 1) * P)
            mch = sm.tile([P, NCH], f32, tag=f"mch{g}")
            ciall = sm.tile([P, NCH], i32, tag=f"ci{g}")
            for cc, (off, w) in enumerate(CHUNKS):
                gu_t = up.tile([P, CW], f32, tag="gu")
                nc.sync.dma_start(gu_t[:, :w], gu_d.ap()[rows, off : off + w])
                lg_t = lp.tile([P, CW], f32, tag="lg")
                nc.sync.dma_start(lg_t[:, :w], lg_d.ap()[rows, off : off + w])
                # ACT: u -> ln(u) -> ln(-ln(u)) = b, in place
                nc.scalar.activation(gu_t[:, :w], gu_t[:, :w], Act.Ln)
                nc.scalar.activation(gu_t[:, :w], gu_t[:, :w], Act.Ln, scale=-1.0)
                # x = lg - b, in place over lg, on gpsimd (DVE only has 2 passes
                # of budget under the DMA roofline: reduce + max_index)
                nc.gpsimd.tensor_tensor(lg_t[:, :w], lg_t[:, :w], gu_t[:, :w], Alu.subtract)
                # chunk row max + within-chunk argmax
                nc.vector.tensor_reduce(
                    mch[:, cc : cc + 1], lg_t[:, :w], mybir.AxisListType.X, Alu.max
                )
                m8 = mp.tile([P, 8], f32, tag="m8")
                nc.vector.tensor_copy(m8[:], mch[:, cc : cc + 1].to_broadcast([P, 8]))
                ci8 = mp.tile([P, 8], u32, tag="ci8")
                nc.vector.max_index(ci8[:], m8[:], lg_t[:, :w])
                nc.vector.tensor_copy(ciall[:, cc : cc + 1], ci8[:, 0:1])

            # ---------------- group endgame: pick winning chunk ----------------
            M8 = sm.tile([P, 8], f32, tag=f"M8{g}")
            Mv = sm.tile([P, 1], f32, tag=f"Mv{g}")
            nc.vector.tensor_reduce(Mv[:], mch[:], mybir.AxisListType.X, Alu.max)
            nc.vector.tensor_copy(M8[:], Mv[:, 0:1].to_broadcast([P, 8]))
            c8 = sm.tile([P, 8], u32, tag=f"c8{g}")
            nc.vector.max_index(c8[:], M8[:], mch[:])
            cst = sm.tile([P, 1], i32, tag=f"cst{g}")
            nc.vector.tensor_copy(cst[:], c8[:, 0:1])
            # masked-sum select of ciall[winner] and basesN[winner]
            eqm = sm.tile([P, NCH], i32, tag=f"eqm{g}")
            nc.vector.tensor_tensor(
                eqm[:], iotaN[:], cst[:, 0:1].to_broadcast([P, NCH]), Alu.is_equal
            )
            selv = sm.tile([P, NCH], i32, tag=f"selv{g}")
            nc.vector.tensor_tensor(selv[:], eqm[:], ciall[:], Alu.mult)
            selb = sm.tile([P, NCH], i32, tag=f"selb{g}")
            nc.vector.tensor_tensor(selb[:], eqm[:], basesN[:], Alu.mult)
            lii = sm.tile([P, 1], i32, tag=f"lii{g}")
            bii = sm.tile([P, 1], i32, tag=f"bii{g}")
            with nc.allow_low_precision(reason="exact int32 sum, single nonzero"):
                nc.vector.tensor_reduce(lii[:], selv[:], mybir.AxisListType.X, Alu.add)
                nc.vector.tensor_reduce(bii[:], selb[:], mybir.AxisListType.X, Alu.add)
            gidx = sm.tile([P, 1], i32, tag=f"gidx{g}")
            nc.vector.tensor_tensor(gidx[:], bii[:], lii[:], Alu.add)

            # ---- gather 1: argmax embedding ----
            v1f = tp.tile([P, 1], f32, tag="v1f")
            nc.vector.tensor_scalar(v1f[:], gidx[:], AV, None, Alu.is_lt)
            s1 = tp.tile([P, 1], f32, tag="s1")
            nc.vector.tensor_tensor(s1[:], v1f[:], s1parts[g][:], Alu.mult)
            idx1c = tp.tile([P, 1], i32, tag="idx1c")
            nc.vector.tensor_scalar(idx1c[:], gidx[:], AV - 1, None, Alu.min)
            e1 = ep.tile([P, D], f32, tag="e1")
            nc.gpsimd.indirect_dma_start(
                out=e1[:],
                out_offset=None,
                in_=w_d.ap(),
                in_offset=bass.IndirectOffsetOnAxis(ap=idx1c[:, 0:1], axis=0),
            )

            # ---- combine + store ----
            o1 = ep.tile([P, D], f32, tag="o1")
            nc.vector.tensor_scalar(o1[:], e1[:], s1[:, 0:1], None, Alu.mult)
            o2 = ep.tile([P, D], f32, tag="o2")
            nc.vector.scalar_tensor_tensor(
                o2[:], e2s[g][:], s2fs[g][:, 0:1], o1[:], Alu.mult, Alu.add
            )
            nc.gpsimd.dma_start(out_d.ap()[rows, :], o2[:])

    nc.compile()
    return nc


def _get_program():
    if "nc" not in _CACHE:
        _CACHE["nc"] = _build_program()
    return _CACHE["nc"]


def make_in_maps(logits, gumbel_u, word_embeddings, rwrt_attention, psg_input):
    lg = np.ascontiguousarray(np.asarray(logits, np.float32).reshape(R, V))
    gu = np.ascontiguousarray(np.asarray(gumbel_u, np.float32).reshape(R, V))
    W = np.ascontiguousarray(np.asarray(word_embeddings, np.float32))
    att = np.ascontiguousarray(np.asarray(rwrt_attention, np.int32))
    psg = np.ascontiguousarray(np.asarray(psg_input, np.int32))
    liota = np.tile(np.arange(L, dtype=np.int32), (B, 1))
    att_flat = att.reshape(R)
    in_maps = []
    for c in range(NCORES):
        r0 = c * RC
        rows = np.arange(r0, r0 + RC, dtype=np.int32)
        in_maps.append(
            {
                "logits": lg[r0 : r0 + RC],
                "gumbel": gu[r0 : r0 + RC],
                "wemb": W,
                "att": att,
                "psg": psg,
                "liota": liota,
                "bcol": np.ascontiguousarray((rows >> 9).reshape(RC, 1)),
                "lcol": np.ascontiguousarray((rows & 511).reshape(RC, 1)),
                "attmy": np.ascontiguousarray(
                    att_flat[r0 : r0 + RC].reshape(RC, 1)
                ),
            }
        )
    return in_maps


def kernel(logits, gumbel_u, word_embeddings, rwrt_attention, psg_input):
    from concourse import bass_utils

    nc = _get_program()
    in_maps = make_in_maps(logits, gumbel_u, word_embeddings, rwrt_attention, psg_input)
    kw = {}
    if os.environ.get("KTRACE"):
        tmpdir = tempfile.mkdtemp(prefix="ktrace_")
        kw = {"trace": True, "tmpdir": tmpdir}
        LAST["tmpdir"] = tmpdir
    res = bass_utils.run_bass_kernel_spmd(
        nc, in_maps, core_ids=list(range(NCORES)), **kw
    )
    LAST["exec_time_ns"] = res.exec_time_ns
    LAST["result"] = res
    out = np.concatenate([res.results[c]["out"] for c in range(NCORES)], axis=0)
    return out.reshape(B, L, D).astype(np.float32)


# revision 9
# speedup vs baseline: 1.1488x; 1.1488x over previous
"""Trainium2 Bass kernel for nn_End2End_10316511445013 (embedding_lookup).

Math being implemented (see the reference nn.Module):
  1. x = logits + g,  g = -ln(-ln(u))          [B,L,V]
  2. In fp32 the straight-through one-hot  y = y_hard + y_soft - y_soft  is
     *exactly* alpha * one_hot(argmax(x)) with alpha = fl(fl(1+s)-s) = 1 +/- 2^-23,
     so the einsum with the embedding table is exactly an embedding row gather
     scaled by alpha (~1, error < 1.2e-7 relative -> we use 1).
  3. inputs_embeds[b,l] = att[b,l] * (idx < AV) * W[idx],  idx = argmax_v x[b,l,:]
  4. psg path: trunc_ids / flag index logic on [B,L] int tensors, then a second
     row gather of W, all computed on-device with small DVE ops + indirect DMA.

Distribution: data-parallel over the B*L = 2048 rows; 256 rows per core; the
94MB embedding table is replicated to every core.  Per core each 128-row
group streams gumbel in chunks (4016 wide, with a narrow 2008/1004/1004
tail to shrink the drain); ACT computes b = Ln(Ln(u)*-1) in place, gpsimd
negates it (1-input op, ~line rate), and the logits chunk is then DMAd
*onto* the tile with accum_op=add (SDMA CCE inline add), yielding
x = logits - b with no DVE pass.  DVE only does the per-chunk row max
(tensor_reduce) and within-chunk argmax (max_index) -- 2 passes, under the
DMA roofline.  Group endgame picks the winning chunk with a [P,NCH]
max_index and masked-sum gathers of the stored per-chunk indices/bases.
"""

import os
import sys
import tempfile

import numpy as np

sys.path.insert(0, "/opt/trn_rl_repo")

B, L, V, AV, D = 4, 512, 32128, 32000, 768
R = B * L            # 2048 tokens total
NCORES = 8
RC = R // NCORES     # 256 tokens per core
P = 128              # partitions
GROUPS = RC // P     # 2 groups of 128 tokens
CW = 4016            # main chunk width
# 7 x 4016 + 2008 + 1004 + 1004 = 32128; narrow tail chunks shrink the drain
CHUNKS = [(k * CW, CW) for k in range(7)] + [
    (7 * CW, 2008),
    (7 * CW + 2008, 1004),
    (7 * CW + 3012, 1004),
]
NCH = len(CHUNKS)    # 10
NEG_BIG = -3.0e38

_CACHE = {}
LAST = {}            # exec_time_ns etc. for test harness introspection


def _build_program():
    from contextlib import ExitStack

    import concourse.bass as bass
    import concourse.tile as tile
    from concourse import bacc, mybir

    f32 = mybir.dt.float32
    i32 = mybir.dt.int32
    u32 = mybir.dt.uint32
    Alu = mybir.AluOpType
    Act = mybir.ActivationFunctionType

    nc = bacc.Bacc(
        "TRN2",
        target_bir_lowering=False,
        debug=False,
        enable_asserts=True,
        num_devices=NCORES,
    )

    lg_d = nc.dram_tensor("logits", [RC, V], f32, kind="ExternalInput")
    gu_d = nc.dram_tensor("gumbel", [RC, V], f32, kind="ExternalInput")
    w_d = nc.dram_tensor("wemb", [AV, D], f32, kind="ExternalInput")
    att_d = nc.dram_tensor("att", [B, L], i32, kind="ExternalInput")
    psg_d = nc.dram_tensor("psg", [B, L], i32, kind="ExternalInput")
    li_d = nc.dram_tensor("liota", [B, L], i32, kind="ExternalInput")
    bc_d = nc.dram_tensor("bcol", [RC, 1], i32, kind="ExternalInput")
    lc_d = nc.dram_tensor("lcol", [RC, 1], i32, kind="ExternalInput")
    am_d = nc.dram_tensor("attmy", [RC, 1], i32, kind="ExternalInput")
    out_d = nc.dram_tensor("out", [RC, D], f32, kind="ExternalOutput")
    sc2_d = nc.dram_tensor("scratch2", [B, 2], i32, kind="Internal")

    att_flat = att_d.ap().rearrange("b (l o) -> (b l) o", o=1)
    psg_flat = psg_d.ap().rearrange("b (l o) -> (b l) o", o=1)

    with tile.TileContext(nc) as tc, ExitStack() as ctx:
        sm = ctx.enter_context(tc.tile_pool(name="small", bufs=1))
        up = ctx.enter_context(tc.tile_pool(name="gu", bufs=4))
        lp = ctx.enter_context(tc.tile_pool(name="lg", bufs=5))
        ep = ctx.enter_context(tc.tile_pool(name="emb", bufs=1))
        tp = ctx.enter_context(tc.tile_pool(name="tok", bufs=2))
        mp = ctx.enter_context(tc.tile_pool(name="mi", bufs=2))

        # ---------------- psg index stage on [B, 512] ----------------
        A_t = sm.tile([B, L], i32, tag="psgA")
        nc.sync.dma_start(A_t[:], att_d.ap())
        P_t = sm.tile([B, L], i32, tag="psgP")
        nc.sync.dma_start(P_t[:], psg_d.ap())
        LI_t = sm.tile([B, L], i32, tag="psgLI")
        nc.sync.dma_start(LI_t[:], li_d.ap())

        shift = sm.tile([B, 1], i32, tag="shift")
        with nc.allow_low_precision(reason="exact int32 sum of 0/1 mask"):
            nc.vector.tensor_reduce(shift[:], A_t[:], mybir.AxisListType.X, Alu.add)

        FA = sm.tile([B, L], i32, tag="FA")  # FA[j] = att[511-j]
        nc.vector.tensor_copy(FA[:], A_t[:, ::-1])
        PR = sm.tile([B, L], i32, tag="PR")  # roll(psg,1) with [:,0]=1
        nc.vector.memset(PR[:, 0:1], 1)
        nc.vector.tensor_copy(PR[:, 1:L], P_t[:, 0 : L - 1])

        t1 = sm.tile([B, L], i32, tag="t1")
        nc.vector.tensor_scalar(t1[:], FA[:], 0, None, Alu.is_equal)
        t2 = sm.tile([B, L], i32, tag="t2")
        nc.vector.tensor_scalar(t2[:], PR[:], 0, None, Alu.not_equal)
        nzm = sm.tile([B, L], i32, tag="nzm")
        nc.vector.tensor_tensor(nzm[:], t1[:], t2[:], Alu.mult)

        # v(j) = (j + shift) & 511 : position in trunc space
        c511b = sm.tile([B, 1], i32, tag="c511b")
        nc.vector.memset(c511b[:], 511)
        v_t = sm.tile([B, L], i32, tag="v")
        nc.vector.tensor_tensor(
            v_t[:], LI_t[:], shift[:, 0:1].to_broadcast([B, L]), Alu.add
        )
        nc.vector.tensor_tensor(
            v_t[:], v_t[:], c511b[:, 0:1].to_broadcast([B, L]), Alu.bitwise_and
        )
        # cand = nz ? v : 9999  ==  (v - 9999)*nz + 9999
        c1 = sm.tile([B, L], i32, tag="c1")
        nc.vector.scalar_tensor_tensor(c1[:], v_t[:], 9999, nzm[:], Alu.subtract, Alu.mult)
        cand = sm.tile([B, L], i32, tag="cand")
        nc.vector.tensor_scalar(cand[:], c1[:], 9999, None, Alu.add)
        nzpos = sm.tile([B, 1], i32, tag="nzpos")
        nc.vector.tensor_reduce(nzpos[:], cand[:], mybir.AxisListType.X, Alu.min)

        s2t = sm.tile([B, 2], i32, tag="s2t")
        nc.vector.tensor_copy(s2t[:, 0:1], shift[:])
        nc.vector.tensor_copy(s2t[:, 1:2], nzpos[:])
        nc.sync.dma_start(sc2_d.ap(), s2t[:])

        ones_i = sm.tile([P, 1], i32, tag="ones")
        nc.vector.memset(ones_i[:], 1)
        c511p = sm.tile([P, 1], i32, tag="c511p")
        nc.vector.memset(c511p[:], 511)
        iotaN = sm.tile([P, NCH], i32, tag="iotaN")
        basesN = sm.tile([P, NCH], i32, tag="basesN")
        for j, (off, _w) in enumerate(CHUNKS):
            nc.vector.memset(iotaN[:, j : j + 1], j)
            nc.vector.memset(basesN[:, j : j + 1], off)

        # ---------------- early psg token-side gathers (independent of phase A) --
        e2s, s2fs, s1parts = [], [], []
        for g in range(GROUPS):
            rows = slice(g * P, (g + 1) * P)
            bvec = tp.tile([P, 1], i32, tag="bvec")
            nc.sync.dma_start(bvec[:], bc_d.ap()[rows, :])
            lvec = tp.tile([P, 1], i32, tag="lvec")
            nc.sync.dma_start(lvec[:], lc_d.ap()[rows, :])
            sn = tp.tile([P, 2], i32, tag="sn")
            nc.gpsimd.indirect_dma_start(
                out=sn[:],
                out_offset=None,
                in_=sc2_d.ap(),
                in_offset=bass.IndirectOffsetOnAxis(ap=bvec[:, 0:1], axis=0),
            )
            # p = (l - shift + 512) & 511
            pv = tp.tile([P, 1], i32, tag="pv")
            nc.vector.tensor_tensor(pv[:], lvec[:], sn[:, 0:1], Alu.subtract)
            nc.vector.tensor_scalar(pv[:], pv[:], 512, None, Alu.add)
            nc.vector.tensor_tensor(pv[:], pv[:], c511p[:], Alu.bitwise_and)
            bsh = tp.tile([P, 1], i32, tag="bsh")
            nc.vector.tensor_scalar(bsh[:], bvec[:], 512, None, Alu.mult)
            # gather att[b, 511-p] : off = b*512 + 511 - p
            offa2 = tp.tile([P, 1], i32, tag="offa2")
            nc.vector.tensor_scalar(offa2[:], pv[:], -1, 511, Alu.mult, Alu.add)
            nc.vector.tensor_tensor(offa2[:], offa2[:], bsh[:], Alu.add)
            gA = tp.tile([P, 1], i32, tag="gA")
            nc.gpsimd.indirect_dma_start(
                out=gA[:],
                out_offset=None,
                in_=att_flat,
                in_offset=bass.IndirectOffsetOnAxis(ap=offa2[:, 0:1], axis=0),
            )
            # gather psg_input[b, p-1] (clamped; p==0 handled by select)
            offp = tp.tile([P, 1], i32, tag="offp")
            nc.vector.tensor_tensor(offp[:], bsh[:], pv[:], Alu.add)
            nc.vector.tensor_scalar(offp[:], offp[:], -1, 0, Alu.add, Alu.max)
            gP = tp.tile([P, 1], i32, tag="gP")
            nc.gpsimd.indirect_dma_start(
                out=gP[:],
                out_offset=None,
                in_=psg_flat,
                in_offset=bass.IndirectOffsetOnAxis(ap=offp[:, 0:1], axis=0),
            )
            eq0 = tp.tile([P, 1], i32, tag="eq0")
            nc.vector.tensor_scalar(eq0[:], pv[:], 0, None, Alu.is_equal)
            gPe = tp.tile([P, 1], i32, tag="gPe")
            nc.vector.select(gPe[:], eq0[:], ones_i[:], gP[:])
            tA = tp.tile([P, 1], i32, tag="tA")
            nc.vector.tensor_scalar(tA[:], gA[:], -1, 1, Alu.mult, Alu.add)
            id2 = tp.tile([P, 1], i32, tag="id2")
            nc.vector.tensor_tensor(id2[:], tA[:], gPe[:], Alu.mult)
            s2f = sm.tile([P, 1], f32, tag=f"s2f{g}")
            nc.vector.tensor_tensor(s2f[:], lvec[:], sn[:, 1:2], Alu.is_ge)
            e2 = sm.tile([P, D], f32, tag=f"e2_{g}")
            nc.gpsimd.indirect_dma_start(
                out=e2[:],
                out_offset=None,
                in_=w_d.ap(),
                in_offset=bass.IndirectOffsetOnAxis(ap=id2[:, 0:1], axis=0),
            )
            am_t = tp.tile([P, 1], i32, tag="am")
            nc.sync.dma_start(am_t[:], am_d.ap()[rows, :])
            attf = sm.tile([P, 1], f32, tag=f"attf{g}")
            nc.vector.tensor_copy(attf[:], am_t[:])
            e2s.append(e2)
            s2fs.append(s2f)
            s1parts.append(attf)

        # ---------------- phase A: stream chunks group-sequentially ----------------
        for g in range(GROUPS):
            rows = slice(g * P, (g + 1) * P)
            mch = sm.tile([P, NCH], f32, tag=f"mch{g}")
            ciall = sm.tile([P, 8 * NCH], u32, tag=f"ci{g}")
            for cc, (off, w) in enumerate(CHUNKS):
                gu_t = up.tile([P, CW], f32, tag="gu")
                nc.sync.dma_start(gu_t[:, :w], gu_d.ap()[rows, off : off + w])
                lg_t = lp.tile([P, CW], f32, tag="lg")
                nc.sync.dma_start(lg_t[:, :w], lg_d.ap()[rows, off : off + w])
                # ACT: u -> ln(u) -> ln(-ln(u)) = b, in place
                nc.scalar.activation(gu_t[:, :w], gu_t[:, :w], Act.Ln)
                nc.scalar.activation(gu_t[:, :w], gu_t[:, :w], Act.Ln, scale=-1.0)
                # x = lg - b, in place over lg, on gpsimd (DVE only has 2 passes
                # of budget under the DMA roofline: reduce + max_index)
                nc.gpsimd.tensor_tensor(lg_t[:, :w], lg_t[:, :w], gu_t[:, :w], Alu.subtract)
                # chunk row max + within-chunk argmax (broadcast AP as in_max;
                # max_index writes its 8 lanes straight into the wide ci tile)
                nc.vector.tensor_reduce(
                    mch[:, cc : cc + 1], lg_t[:, :w], mybir.AxisListType.X, Alu.max
                )
                nc.vector.max_index(
                    ciall[:, 8 * cc : 8 * cc + 8],
                    mch[:, cc : cc + 1].to_broadcast([P, 8]),
                    lg_t[:, :w],
                )

            # ---------------- group endgame: pick winning chunk ----------------
            M8 = sm.tile([P, 8], f32, tag=f"M8{g}")
            Mv = sm.tile([P, 1], f32, tag=f"Mv{g}")
            nc.vector.tensor_reduce(Mv[:], mch[:], mybir.AxisListType.X, Alu.max)
            nc.vector.tensor_copy(M8[:], Mv[:, 0:1].to_broadcast([P, 8]))
            c8 = sm.tile([P, 8], u32, tag=f"c8{g}")
            nc.vector.max_index(c8[:], M8[:], mch[:])
            cst = sm.tile([P, 1], i32, tag=f"cst{g}")
            nc.vector.tensor_copy(cst[:], c8[:, 0:1])
            # masked-sum select of ciall[winner] and basesN[winner]
            eqm = sm.tile([P, NCH], i32, tag=f"eqm{g}")
            nc.vector.tensor_tensor(
                eqm[:], iotaN[:], cst[:, 0:1].to_broadcast([P, NCH]), Alu.is_equal
            )
            selv = sm.tile([P, NCH], i32, tag=f"selv{g}")
            nc.vector.tensor_tensor(selv[:], eqm[:], ciall[:, :: 8], Alu.mult)
            selb = sm.tile([P, NCH], i32, tag=f"selb{g}")
            nc.vector.tensor_tensor(selb[:], eqm[:], basesN[:], Alu.mult)
            lii = sm.tile([P, 1], i32, tag=f"lii{g}")
            bii = sm.tile([P, 1], i32, tag=f"bii{g}")
            with nc.allow_low_precision(reason="exact int32 sum, single nonzero"):
                nc.vector.tensor_reduce(lii[:], selv[:], mybir.AxisListType.X, Alu.add)
                nc.vector.tensor_reduce(bii[:], selb[:], mybir.AxisListType.X, Alu.add)
            gidx = sm.tile([P, 1], i32, tag=f"gidx{g}")
            nc.vector.tensor_tensor(gidx[:], bii[:], lii[:], Alu.add)

            # ---- gather 1: argmax embedding ----
            v1f = tp.tile([P, 1], f32, tag="v1f")
            nc.vector.tensor_scalar(v1f[:], gidx[:], AV, None, Alu.is_lt)
            s1 = tp.tile([P, 1], f32, tag="s1")
            nc.vector.tensor_tensor(s1[:], v1f[:], s1parts[g][:], Alu.mult)
            idx1c = tp.tile([P, 1], i32, tag="idx1c")
            nc.vector.tensor_scalar(idx1c[:], gidx[:], AV - 1, None, Alu.min)
            e1 = ep.tile([P, D], f32, tag="e1")
            nc.gpsimd.indirect_dma_start(
                out=e1[:],
                out_offset=None,
                in_=w_d.ap(),
                in_offset=bass.IndirectOffsetOnAxis(ap=idx1c[:, 0:1], axis=0),
            )

            # ---- combine + store ----
            o1 = ep.tile([P, D], f32, tag="o1")
            nc.vector.tensor_scalar(o1[:], e1[:], s1[:, 0:1], None, Alu.mult)
            o2 = ep.tile([P, D], f32, tag="o2")
            nc.vector.scalar_tensor_tensor(
                o2[:], e2s[g][:], s2fs[g][:, 0:1], o1[:], Alu.mult, Alu.add
            )
            nc.gpsimd.dma_start(out_d.ap()[rows, :], o2[:])

    nc.compile()
    return nc


def _get_program():
    if "nc" not in _CACHE:
        _CACHE["nc"] = _build_program()
    return _CACHE["nc"]


def make_in_maps(logits, gumbel_u, word_embeddings, rwrt_attention, psg_input):
    lg = np.ascontiguousarray(np.asarray(logits, np.float32).reshape(R, V))
    gu = np.ascontiguousarray(np.asarray(gumbel_u, np.float32).reshape(R, V))
    W = np.ascontiguousarray(np.asarray(word_embeddings, np.float32))
    att = np.ascontiguousarray(np.asarray(rwrt_attention, np.int32))
    psg = np.ascontiguousarray(np.asarray(psg_input, np.int32))
    liota = np.tile(np.arange(L, dtype=np.int32), (B, 1))
    att_flat = att.reshape(R)
    in_maps = []
    for c in range(NCORES):
        r0 = c * RC
        rows = np.arange(r0, r0 + RC, dtype=np.int32)
        in_maps.append(
            {
                "logits": lg[r0 : r0 + RC],
                "gumbel": gu[r0 : r0 + RC],
                "wemb": W,
                "att": att,
                "psg": psg,
                "liota": liota,
                "bcol": np.ascontiguousarray((rows >> 9).reshape(RC, 1)),
                "lcol": np.ascontiguousarray((rows & 511).reshape(RC, 1)),
                "attmy": np.ascontiguousarray(
                    att_flat[r0 : r0 + RC].reshape(RC, 1)
                ),
            }
        )
    return in_maps


def kernel(logits, gumbel_u, word_embeddings, rwrt_attention, psg_input):
    from concourse import bass_utils

    nc = _get_program()
    in_maps = make_in_maps(logits, gumbel_u, word_embeddings, rwrt_attention, psg_input)
    kw = {}
    if os.environ.get("KTRACE"):
        tmpdir = tempfile.mkdtemp(prefix="ktrace_")
        kw = {"trace": True, "tmpdir": tmpdir}
        LAST["tmpdir"] = tmpdir
    res = bass_utils.run_bass_kernel_spmd(
        nc, in_maps, core_ids=list(range(NCORES)), **kw
    )
    LAST["exec_time_ns"] = res.exec_time_ns
    LAST["result"] = res
    out = np.concatenate([res.results[c]["out"] for c in range(NCORES)], axis=0)
    return out.reshape(B, L, D).astype(np.float32)
